# revision 31
# baseline (speedup 1.0000x reference)
"""Bilateral filter (d=5, sigmaColor=0.1, sigmaSpace=1) Trainium2 Bass kernel.

Full inputs in, full outputs out. Data-parallel over 8 NeuronCores: 2 images
per core. Per-core layout: partitions = (img, row-block-of-8); each partition
stores, in fp16, a planar [ch][12 rows][Wp+4 cols] center tile whose 2-row /
2-col halos make every 5x5 window offset a pure free-dim AP shift.

v5 (CoreSim 257us vs v2 baseline 305us): difference-form output + engine
rebalance + cross-pass software pipelining.

Math: out = I + T/den with T = sum_o w_o(p)*(I(p+o)-I(p)), den = 1 + sum w.
  For a mirror pair {o,-o} the contribution to T is G_o(p-o) - G_o(p) with a
  SINGLE product field G_o = w_o * D_o per pair (D_o = ctr - nbr already
  exists for the range kernel), halving product work vs the num/den form and
  dropping the center-tap matmuls. Negated accumulation uses a -identity
  stationary matrix on PE. Accuracy improves vs v2 (max abs err 3.9e-4 vs
  6.7e-4): the dominant I term no longer round-trips through fp16 products.

Engine assignment (balanced ~47us/engine/pass): diff on DVE; squares split
  ACT/DVE/Pool (SQ_ENG); channel-sum Pool (+1 pair DVE); exp on ACT (spatial
  weight in bias); G-products split DVE/Pool (PROD_DVE); accumulation on PE
  into fp32 PSUM (T 3 banks + den 1 per row-half, 8 banks total). Input is
  staged f32 via sync/scalar HWDGE DMAs (no Pool SWDGE casting) and cast to
  fp16 inside the ACT deinterleave copy. Drain: den+1 (ACT bias) -> fast
  reciprocal (DVE) -> T*rden (DVE, from PSUM) -> +I (Pool) -> interleaved
  f32 store.

Schedule: 4 column passes (Wp=128).
  - Cross-pass product pipeline: the last LOOKAHEAD pairs'"'"' G-products and
    T-matmuls of pass k are emitted interleaved with pass k+1'"'"'s first weight
    chains, so no engine queue head-blocks at the pass boundary.
  - exp is emitted one pair late (exp_q) so ACT never stalls on Pool'"'"'s csum;
    den matmuls ride with exp (they only need w), closing den'"'"'s PSUM
    accumulation ~LOOKAHEAD pairs before T'"'"'s so the den drain never waits.
  - Pass k+1'"'"'s loads issue at pass k'"'"'s top; its deinterleave runs mid-pass k
    (DEINT_AT) so boundary diffs start immediately.
  - Pass 0 fans load issue across sync+scalar queues (HWDGE issue is ~2.4us
    per DMA, serial per queue).
  - drain(k) is emitted at pass k+1'"'"'s oi==LOOKAHEAD, after PSUM den/T tag
    reuse points.
"""

import os
import sys

import numpy as np

for _p in ("/opt/trn_rl_repo",):
    if os.path.isdir(_p) and _p not in sys.path:
        sys.path.append(_p)

import concourse.bacc as bacc
import concourse.bass as bass
import concourse.tile as tile
from concourse import masks, mybir
from concourse.ap import AP
from concourse.bass_utils import run_bass_kernel_spmd

F16 = mybir.dt.float16
F32 = mybir.dt.float32
ALU = mybir.AluOpType
ACTF = mybir.ActivationFunctionType

N_CORES = 8
R = 2  # window radius

# 12 symmetric pair representatives: dy > 0, or dy == 0 and dx > 0.
PAIRS = [(0, 1), (0, 2)] + [(dy, dx) for dy in (1, 2) for dx in range(-R, R + 1)]

# Engine assignment tunables (balance DVE/ACT/Pool busy time).
SQ_ENG = {0: "dve", 4: "dve", 8: "dve", 6: "pool", 10: "pool"}  # default "act"
CSUM_ENG = {1: "dve", 5: "dve", 9: "dve"}   # default "pool"
DEINT_AT = 8  # pair index at which the next pass's deinterleave is emitted
PROD_DVE = {1, 3, 5, 7, 9, 11}                           # G-product on DVE, else Pool
LOOKAHEAD = 6


class Cfg:
    def __init__(self, B=2, H=512, W=512, Wp=128):
        self.B, self.H, self.W, self.Wp = B, H, W, Wp
        self.C = 3
        self.RBR = 8                      # core rows per partition
        self.RBN = H // self.RBR          # row blocks per image
        self.P = B * self.RBN             # partitions
        self.RH = self.RBR + 2 * R        # stored rows (12)
        self.WS = Wp + 2 * R              # stored cols per pass
        self.NPASS = W // Wp
        assert H % self.RBR == 0 and W % Wp == 0 and self.P <= 128
        assert self.WS % 2 == 0


FULL = Cfg()


def build(cfg: Cfg, enable_asserts=False, repeat=1):
    B, H, W, Wp, C = cfg.B, cfg.H, cfg.W, cfg.Wp, cfg.C
    P, RBN, RBR, RH, WS = cfg.P, cfg.RBN, cfg.RBR, cfg.RH, cfg.WS
    WC = W * C          # f32 elems per image row in DRAM
    HWC = H * WC
    RH4 = RBR // 2      # rows per PSUM row-half (4)
    NP = len(PAIRS)

    nc = bacc.Bacc(
        "TRN2",
        target_bir_lowering=False,
        debug=False,
        enable_asserts=enable_asserts,
        num_devices=N_CORES,
    )
    # constant biases for the fused exp (one per distinct spatial distance)
    for bv in sorted({-0.5 * float(dy * dy + dx * dx) for dy, dx in PAIRS}):
        t = nc.alloc_sbuf_tensor(f"const-bias-{bv}", [128, 1], F32)
        nc.gpsimd.memset(t.ap(), bv)
        nc.const_aps.aps[(F32, bv)] = t.ap()
    ident = nc.alloc_sbuf_tensor("ident", [128, 128], F16)
    negid = nc.alloc_sbuf_tensor("negid", [128, 128], F16)
    nc.vector.memset(ident.ap(), 0.0)
    nc.all_engine_barrier()
    masks.make_identity(nc, ident.ap(), nomemset=True)
    nc.all_engine_barrier()
    nc.vector.tensor_scalar_mul(negid.ap(), ident.ap(), -1.0)
    nc.all_engine_barrier()

    x_h = nc.dram_tensor("x", [B, H, W, C], F32, kind="ExternalInput")
    y_h = nc.dram_tensor("out", [B, H, W, C], F32, kind="ExternalOutput")
    x_flat = x_h.ap().rearrange("b h w c -> (b h w c)")
    y_flat = y_h.ap().rearrange("b h w c -> (b h w c)")
    id_ap = ident.ap()
    nid_ap = negid.ap()

    def dram_ap(flat, offset, dims):
        return AP(flat.tensor, offset, [list(d) for d in dims])

    with tile.TileContext(nc) as tc:
        with (
            tc.tile_pool(name="state", bufs=1) as state_pool,
            tc.tile_pool(name="big", bufs=2) as big_pool,
            tc.tile_pool(name="small", bufs=2) as small_pool,
            tc.tile_pool(name="psum", bufs=1, space="PSUM") as psum_pool,
        ):
            WS2 = Wp + 4   # chunk tile cols: pass cols + 2-col halos
            zt = state_pool.tile([P, 2 * WS2 * C], F32, name="zt", tag="zt")
            nc.vector.memset(zt[:, :], 0.0)
            drain_prev = None
            drain_den_prev = None
            pending = []
            exp_q = []

            def emit_load_dmas(ps, multi_queue=False):
                ps0 = ps % cfg.NPASS
                cl = ps0 * Wp
                v_lo = max(0, cl - R)
                v_hi = min(W, cl + Wp + R)
                nv = v_hi - v_lo
                nvC = nv * C
                s_lo = v_lo - cl + R   # stored col of first loaded col
                Ct = state_pool.tile([P, C, RH, WS2], F16, name=f"C_{ps}", tag=f"C{ps0}")
                if s_lo > 0:
                    nc.vector.memset(Ct[:, :, :, 0:s_lo], 0.0)
                if s_lo + nv < WS2:
                    nc.vector.memset(Ct[:, :, :, s_lo + nv : WS2], 0.0)
                # startup fans the load issue across engine queues (HWDGE
                # issue is ~2.4us per DMA, serial per queue); steady passes
                # hide their load issue under compute on sync alone.
                qs = [nc.sync, nc.scalar] if multi_queue else [nc.sync]
                qi = [0]

                def ld_q():
                    q = qs[qi[0] % len(qs)]
                    qi[0] += 1
                    return q

                deints = []
                for ck in range(2):  # stored rows [6*ck, 6*ck+6)
                    r0 = 6 * ck
                    St = big_pool.tile([P, 6 * nvC], F32, name=f"S_{ps}_{ck}", tag="S", bufs=2)
                    # image row of stored row r is 8*rb - 2 + r
                    rb_a = 1 if ck == 0 else 0
                    rb_b = RBN if ck == 0 else RBN - 1
                    for img in range(B):
                        pb = img * RBN
                        row0 = 8 * rb_a - 2 + r0
                        ld_q().dma_start(
                            out=St[pb + rb_a : pb + rb_b, :].rearrange(
                                "p (r w) -> p r w", r=6, w=nvC
                            ),
                            in_=dram_ap(
                                x_flat,
                                img * HWC + row0 * WC + v_lo * C,
                                [(8 * WC, rb_b - rb_a), (WC, 6), (1, nvC)],
                            ),
                        )
                        if ck == 0:
                            # rb=0: stored rows 0..2 above the image -> 0
                            ld_q().dma_start(
                                out=St[pb : pb + 1, 0 : 2 * nvC],
                                in_=zt[pb : pb + 1, 0 : 2 * nvC],
                            )
                            # rb=0: stored rows 2..6 <- image rows 0..4
                            ld_q().dma_start(
                                out=St[pb : pb + 1, 2 * nvC : 6 * nvC].rearrange(
                                    "p (r w) -> p r w", r=4, w=nvC
                                ),
                                in_=dram_ap(
                                    x_flat,
                                    img * HWC + v_lo * C,
                                    [(8 * WC, 1), (WC, 4), (1, nvC)],
                                ),
                            )
                        else:
                            # rb=RBN-1: rows 10..12 below the image -> 0
                            pe_ = pb + RBN - 1
                            ld_q().dma_start(
                                out=St[pe_ : pe_ + 1, 4 * nvC : 6 * nvC],
                                in_=zt[pe_ : pe_ + 1, 0 : 2 * nvC],
                            )
                            # rb=RBN-1: rows 6..10 <- image rows H-4..H
                            ld_q().dma_start(
                                out=St[pe_ : pe_ + 1, 0 : 4 * nvC].rearrange(
                                    "p (r w) -> p r w", r=4, w=nvC
                                ),
                                in_=dram_ap(
                                    x_flat,
                                    img * HWC + (H - 4) * WC + v_lo * C,
                                    [(8 * WC, 1), (WC, 4), (1, nvC)],
                                ),
                            )
                    # deinterleave + cast: C[ch, r0+r, s_lo+w] = S[r, w, ch]
                    s_v = St[:, :].rearrange("p (r w c) -> p c r w", r=6, w=nv, c=C)
                    deints.append((Ct, r0, s_lo, nv, s_v))

                def deint():
                    for Ct_, r0, s_lo, nv, s_v in deints:
                        nc.scalar.copy(Ct_[:, :, r0 : r0 + 6, s_lo : s_lo + nv], s_v)

                return Ct, deint

            cur = emit_load_dmas(0, multi_queue=True)
            cur[1]()   # pass-0 deint right away
            for rep in range(repeat):
                for ps0 in range(cfg.NPASS):
                    ps = rep * cfg.NPASS + ps0
                    cl = ps0 * Wp       # first image col of this pass (even)
                    Ct = cur[0]
                    nxt = None
                    if ps + 1 < repeat * cfg.NPASS:
                        nxt = emit_load_dmas(ps + 1)
                    if drain_den_prev is not None:
                        drain_den_prev()
                        drain_den_prev = None

                    T_ps = None   # allocated after drain_prev is emitted
                    den_ps = [
                        psum_pool.tile([P, RH4 * Wp], F32, name=f"dps_{ps}_{rh}", tag=f"den{rh}")
                        for rh in range(2)
                    ]

                    def emit_exp_den(ent):
                        eoi, ew, ecd, edy, edx, ecwA, ecwB = ent
                        nc.scalar.activation(
                            ew[:, :, :], ecd[:, :, :], ACTF.Exp,
                            bias=-0.5 * float(edy * edy + edx * edx), scale=-50.0,
                        )
                        wA = ew[:, edy : edy + RBR, ecwA : ecwA + Wp]
                        wB = ew[:, 0:RBR, ecwB : ecwB + Wp]
                        for rh in range(2):
                            rr = RH4 * rh
                            mm(den_ps[rh], id_ap[0:P, 0:P], wA[:, rr : rr + RH4, :], first=(eoi == 0))
                            mm(den_ps[rh], id_ap[0:P, 0:P], wB[:, rr : rr + RH4, :],
                               first=False, last=(eoi == NP - 1))

                    def mm(psum_flat, lhs, rhs_view, first, last=False):
                        nc.tensor.matmul(
                            psum_flat, lhs, rhs_view,
                            start=first, stop=last,
                        )

                    ps_ref = {}

                    def emit_products(ent):
                        oi, dy, dx, w, D, Re, We, cwA, cwB, ref = ent
                        T_ps, den_ps = ref["T"], ref["den"]
                        G = big_pool.tile([P, C, Re, We], F16, name=f"G_{ps}_{oi}", tag="G", bufs=3)
                        peng = nc.vector if oi in PROD_DVE else nc.gpsimd
                        peng.tensor_mul(
                            G[:, :, :, :],
                            w.unsqueeze(1).broadcast_to((P, C, Re, We)),
                            D[:, :, :, :],
                        )
                        for rh in range(2):
                            rr = RH4 * rh
                            for c in range(C):
                                tpl = T_ps[rh][:, c, :, :].rearrange("p r w -> p (r w)")
                                # +G(p-o): stored rows rr..rr+4, cols cwB..
                                gB = G[:, c, rr : rr + RH4, cwB : cwB + Wp]
                                # -G(p): stored rows dy+rr.., cols cwA..
                                gA = G[:, c, dy + rr : dy + rr + RH4, cwA : cwA + Wp]
                                mm(tpl, id_ap[0:P, 0:P], gB, first=(oi == 0))
                                mm(tpl, nid_ap[0:P, 0:P], gA, first=False,
                                   last=(oi == NP - 1))

                    for oi, (dy, dx) in enumerate(PAIRS):
                        r0 = R - dy
                        Re = RBR + dy
                        col_lo = (R - max(dx, 0)) & ~1
                        col_hi = R + Wp - min(dx, 0)
                        if (col_hi - col_lo) % 2:
                            col_hi += 1
                        We = col_hi - col_lo
                        gl = col_lo           # chunk-local stored col of w col 0

                        ctr_e = Ct[:, :, r0 : r0 + Re, gl : gl + We]
                        gb = gl + dx
                        nbr_e = Ct[:, :, r0 + dy : r0 + dy + Re, gb : gb + We]

                        D = big_pool.tile([P, C, Re, We], F16, name=f"D_{ps}_{oi}", tag="D", bufs=8)
                        SQ = big_pool.tile([P, C, Re, We], F16, name=f"SQ_{ps}_{oi}", tag="SQ", bufs=2)
                        cd = small_pool.tile([P, Re, We], F16, name=f"cd_{ps}_{oi}", tag="cd", bufs=2)
                        w = small_pool.tile([P, Re, We], F16, name=f"w_{ps}_{oi}", tag="w", bufs=7)

                        nc.vector.tensor_sub(D[:, :, :, :], ctr_e, nbr_e)
                        sq_eng = SQ_ENG.get(oi, "act")
                        if sq_eng == "act":
                            nc.scalar.activation(SQ[:, :, :, :], D[:, :, :, :], ACTF.Square)
                        elif sq_eng == "dve":
                            nc.vector.tensor_mul(SQ[:, :, :, :], D[:, :, :, :], D[:, :, :, :])
                        else:
                            nc.gpsimd.tensor_mul(SQ[:, :, :, :], D[:, :, :, :], D[:, :, :, :])
                        cs_eng = nc.vector if CSUM_ENG.get(oi, "pool") == "dve" else nc.gpsimd
                        cs_eng.tensor_tensor(cd[:, :, :], SQ[:, 0], SQ[:, 1], ALU.add)
                        cs_eng.tensor_tensor(cd[:, :, :], cd[:, :, :], SQ[:, 2], ALU.add)
                        # column (in chain tiles) of the first center pixel
                        cwA = R - col_lo          # w col for A-side (w at p)
                        cwB = R - dx - col_lo     # w col for B-side (w at p-o)
                        # exp is emitted one pair late (see exp_q) so ACT's
                        # in-order queue never stalls waiting on Pool's csum.
                        # den matmuls ride with exp: they only need w, so den's
                        # accumulation closes ~LOOKAHEAD pairs before T's and
                        # the den drain (denf -> recip) never stalls on PE.
                        exp_q.append((oi, w, cd, dy, dx, cwA, cwB))
                        if len(exp_q) > 1:
                            emit_exp_den(exp_q.pop(0))
                        pending.append((oi, dy, dx, w, D, Re, We, cwA, cwB, ps_ref))

                        if oi == LOOKAHEAD:
                            # cross-pass pipelining: the previous pass's drain
                            # lands here, after this pass's first weight
                            # chains are queued, then PSUM is (re)claimed.
                            if drain_prev is not None:
                                drain_prev()
                                drain_prev = None
                            ps_ref["T"] = T_ps = [
                                psum_pool.tile([P, C, RH4, Wp], F32, name=f"tps_{ps}_{rh}", tag=f"num{rh}")
                                for rh in range(2)
                            ]
                            ps_ref["den"] = den_ps
                        if oi == DEINT_AT and nxt is not None:
                            nxt[1]()
                        if len(pending) > LOOKAHEAD:
                            emit_products(pending.pop(0))
                    while exp_q:
                        emit_exp_den(exp_q.pop(0))
                    cur = nxt

                    rden_box = {}

                    def make_drain_den(ps=ps, den_ps=den_ps, rden_box=rden_box):
                        def drain_den():
                            for rh in range(2):
                                denf = small_pool.tile([P, RH4 * Wp], F32, name=f"denf_{ps}_{rh}", tag="denf", bufs=1)
                                rden = small_pool.tile([P, RH4 * Wp], F32, name=f"rden_{ps}_{rh}", tag=f"rden{rh}", bufs=1)
                                nc.scalar.activation(denf[:, :], den_ps[rh][:, :], ACTF.Copy, bias=1.0)
                                nc.vector.reciprocal_approx_fast(rden[:, :], denf[:, :])
                                rden_box[rh] = rden
                        return drain_den

                    def make_drain(ps=ps, cl=cl, Ct=Ct, T_ps=T_ps, rden_box=rden_box):
                        def drain():
                            for rh in range(2):
                                rr = RH4 * rh
                                rden = rden_box[rh]
                                rb3 = rden.rearrange("p (r w) -> p r w", r=RH4, w=Wp)
                                Mt = small_pool.tile([P, C * RH4 * Wp], F32, name=f"M_{ps}_{rh}", tag="M", bufs=1)
                                m_v = Mt[:, :].rearrange("p (c r w) -> p c r w", r=RH4, w=Wp, c=C)
                                nc.vector.tensor_mul(
                                    m_v[:, :, :, :],
                                    T_ps[rh][:, :, :, :],
                                    rb3.unsqueeze(1).broadcast_to((P, C, RH4, Wp)),
                                )
                                Oi = small_pool.tile([P, RH4 * Wp * C], F32, name=f"Oi_{ps}_{rh}", tag="Oi", bufs=2)
                                o_v = Oi[:, :].rearrange("p (r w c) -> p c r w", r=RH4, w=Wp, c=C)
                                # out = I + T/den, straight into the
                                # interleaved f32 store layout
                                nc.gpsimd.tensor_tensor(
                                    o_v[:, :, :, :],
                                    m_v[:, :, :, :],
                                    Ct[:, :, R + rr : R + rr + RH4, R : R + Wp],
                                    ALU.add,
                                )
                                for img in range(B):
                                    pb = img * RBN
                                    nc.sync.dma_start(
                                        out=dram_ap(
                                            y_flat,
                                            img * HWC + rr * WC + cl * C,
                                            [(8 * WC, RBN), (WC, RH4), (1, Wp * C)],
                                        ),
                                        in_=Oi[pb : pb + RBN, :].rearrange(
                                            "p (r w) -> p r w", r=RH4, w=Wp * C
                                        ),
                                    )
                        return drain

                    drain_prev = make_drain()
                    drain_den_prev = make_drain_den()
            drain_den_prev()
            for ent in pending:
                emit_products(ent)
            pending.clear()
            drain_prev()

    nc.compile()
    return nc


def make_timed_fn(nc, in_maps, n_cores=N_CORES):
    """Jitted sharded executor over device-resident inputs, no donation
    (kernel writes every output element), for wall-clock benchmarking."""
    import jax
    from jax.sharding import Mesh, PartitionSpec
    from jax.experimental.shard_map import shard_map
    import concourse.bass2jax as b2j
    from concourse import mybir as _mb

    b2j.install_neuronx_cc_hook()
    partition_name = nc.partition_id_tensor.name if nc.partition_id_tensor else None
    in_names, out_names, out_avals = [], [], []
    for alloc in nc.m.functions[0].allocations:
        if not isinstance(alloc, _mb.MemoryLocationSet):
            continue
        name = alloc.memorylocations[0].name
        if alloc.kind == "ExternalInput":
            if name != partition_name:
                in_names.append(name)
        elif alloc.kind == "ExternalOutput":
            out_names.append(name)
            out_avals.append(
                jax.core.ShapedArray(tuple(alloc.tensor_shape), _mb.dt.np(alloc.dtype))
            )
    n_params = len(in_names)
    zero_outs = [np.zeros(a.shape, a.dtype) for a in out_avals]
    all_in_names = list(in_names) + list(out_names)
    if partition_name is not None:
        all_in_names.append(partition_name)
    if nc.dbg_addr is not None:
        in_maps = [
            {**m, nc.dbg_addr.name: np.zeros((1, 2), np.uint32)} for m in in_maps
        ]
        if nc.dbg_addr.name not in in_names:
            in_names.append(nc.dbg_addr.name)
            all_in_names.insert(len(in_names) - 1, nc.dbg_addr.name)
            n_params += 1

    def _body(*args):
        operands = list(args)
        if partition_name is not None:
            operands.append(b2j.partition_id_tensor())
        return tuple(
            b2j._bass_exec_p.bind(
                *operands,
                out_avals=tuple(out_avals),
                in_names=tuple(all_in_names),
                out_names=tuple(out_names),
                lowering_input_output_aliases=(),
                sim_require_finite=True,
                sim_require_nnan=True,
                nc=nc,
            )
        )

    devices = jax.devices()[:n_cores]
    mesh = Mesh(np.asarray(devices), ("core",))
    n_outs = len(out_names)
    sharded = jax.jit(
        shard_map(
            _body,
            mesh=mesh,
            in_specs=(PartitionSpec("core"),) * (n_params + n_outs),
            out_specs=(PartitionSpec("core"),) * n_outs,
            check_rep=False,
        ),
        keep_unused=True,
    )
    concat_in = [
        np.concatenate([np.asarray(m[name]) for m in in_maps], axis=0)
        for name in in_names
    ]
    concat_zero = [
        np.zeros((n_cores * z.shape[0], *z.shape[1:]), z.dtype) for z in zero_outs
    ]
    sharding = jax.sharding.NamedSharding(mesh, PartitionSpec("core"))
    dev_args = [jax.device_put(a, sharding) for a in concat_in + concat_zero]

    def run():
        outs = sharded(*dev_args)
        jax.block_until_ready(outs)
        return outs

    return run


def bench(x=None, iters=6, repeats=(1, 5)):
    import time as _t

    if x is None:
        rng = np.random.default_rng(0)
        x = rng.random((16, 512, 512, 3), dtype=np.float32)
    x = np.ascontiguousarray(np.asarray(x), np.float32)
    bpc = x.shape[0] // N_CORES
    in_maps = [{"x": x[i * bpc : (i + 1) * bpc]} for i in range(N_CORES)]
    times = {}
    for rep in repeats:
        nc = build(FULL, repeat=rep)
        fn = make_timed_fn(nc, in_maps)
        fn()  # compile + warmup
        fn()
        ts = []
        for _ in range(iters):
            t0 = _t.perf_counter()
            fn()
            ts.append(_t.perf_counter() - t0)
        times[rep] = min(ts)
        print(f"repeat={rep}: min wall {times[rep]*1e6:.0f} us over {iters} iters")
    r0, r1 = repeats
    hw_ns = (times[r1] - times[r0]) / (r1 - r0) * 1e9
    print(f"HW exec time: {hw_ns:.0f} ns")
    return hw_ns


_NC_CACHE = {}


def _get_nc():
    if "full" not in _NC_CACHE:
        _NC_CACHE["full"] = build(FULL)
    return _NC_CACHE["full"]


def kernel(x, trace=False, **_ignored):
    x = np.ascontiguousarray(np.asarray(x), dtype=np.float32)
    B = x.shape[0]
    bpc = B // N_CORES
    nc = _get_nc()
    in_maps = [{"x": x[i * bpc : (i + 1) * bpc]} for i in range(N_CORES)]
    res = run_bass_kernel_spmd(nc, in_maps, list(range(N_CORES)), trace=trace)
    out = np.concatenate([res.results[i]["out"] for i in range(N_CORES)], axis=0)
    if trace:
        kernel.last_results = res
    return out.astype(np.float32)
